# revision 24
# baseline (speedup 1.0000x reference)
"""Trainium2 Bass kernel for nn_DendriticLayer.

Reference computation (all fp32 in DRAM):
    h   = leaky(x @ (Wd * dendrite_mask).T + bd)   # [B, N_SOMA]
    out = leaky(h @ (Ws * soma_mask).T + bs)       # [B, N_NEURONS]
with leaky(z) = where(z >= 0, z, 0.1 z).

Structure exploited:
  * soma_mask is block-diagonal: neuron n reads only its 16 contiguous
    dendrites, so stage 2 is a grouped contraction done as accumulating
    [128x128] matmuls on zero-padded block weights.
  * Sharding: somas (and their neurons) split 8 ways; core c computes
    h for somas [2048c, 2048c+2048) and out for neurons [128c, 128c+128).
    No cross-core communication.

v2 design notes (vs the first working version, HW-validated):
  * All DMAs are large and per-partition contiguous: the host pre-arranges
    DRAM layouts (pure layout transforms) so x ships as 1-2 DMAs per
    1024-wide batch block and the weights as quarter DMAs, instead of
    100+ 1KB-per-partition strided DMAs (descriptor-bound: the old cold
    phase was ~170us of the 279us total exec).
  * mm_mode="hyb" (available; default is bf16 - the PE pays ~0.5us per
    weight-mode switch, which ate the fp8 gain): k-chunks 0..3 as fp8
    (float8_e4m3,
    TRN range) DoubleRow matmuls - two k-chunks per PE instruction at
    ~1.5x bf16 rate - and k-chunks 4..7 as bf16.  Full fp8 fails the
    2e-2 gate (rel err 2.02e-2); half fp8 measures 1.392e-2 on HW.  All
    Wd ships pre-scaled by 32 (exact power of two) so fp8 stays in
    normal range and both halves share one PSUM scale; the stage-1 ACT
    eviction compensates with scale=1/32.  The dendrite mask is 0/1 =>
    exact in fp8; masking still runs on device (DVE).  Per chunk all 4
    DoubleRow matmuls issue before all 8 bf16 ones - same-mode batching
    halves PE weight-mode switches.  mm_mode="bf16"/"fp8" keep the same
    structure single-dtype.
  * PSUM stays strictly single-bank per tensor and ACT reads never cross
    a bank: [128,1024] two-bank PSUM buffers with paired ACT evicts
    benched 3x SLOWER (332us/pass) and col-tiled M=32 stage-2 matmuls
    (tile_position) benched ~+190us/pass - both reverted.
  * x staging copy (DMA -> xall -> DVE copy -> xc) is kept; the fp8 copy
    runs as a uint32-bitcast copy (4x fewer DVE elements).

This build accepts only ONE semaphore wait per engine instruction, so the
kernel is raw Bass: every cross-engine dependency is a standalone wait_ge
on the consuming engine, with semaphore values precomputed by a static
planner.  DMA groups get parity semaphores; consumers wait whole groups.

Host-side input preparation is layout + dtype-quantization only
(transpose / reshape / slice / cast); every arithmetic op of the
reference runs on device.
"""

import numpy as np

N_CORES = 8
B = 4096
IN_DIM = 1024
N_SOMA = 16384
N_NEURONS = 1024
ND = 16                      # dendrites per neuron
P = 128
S_SH = N_SOMA // N_CORES     # 2048 somas per core
N_SH = N_NEURONS // N_CORES  # 128 neurons per core
NCH = S_SH // P              # 16 soma chunks of 128
KT = IN_DIM // P             # 8 contraction chunks of 128
KT2 = KT // 2                # 4 DoubleRow pair-chunks
BBLK = 1024                  # batch block (2 PSUM banks)
NB = B // BBLK               # 4 batch blocks
NQ = 4                       # weight-column quarters (512 somas each)
WPC = S_SH // NQ             # 512 somas per quarter
SLOPE = 0.1
NH = 6                       # hT buffers
K_TOT = NB * NCH             # 64 (bb, cc) chunks per pass

_PROGRAM_CACHE: dict = {}


def _streams(repeat: int = 1, hyb: bool = False):
    """Single source of truth for all four engine instruction streams.

    Returns {engine: [op, ...]} where ops are tuples:
      ("wait", event)            wait until the event's semaphore value
      ("<op>", *args, event)     instruction; event names its sem inc
    """
    nbt = repeat * NB  # total batch blocks (same data re-processed when >NB)
    k_tot = nbt * NCH

    sp = []
    whiches = ("wd8", "dm8", "wd16", "dm16") if hyb else ("wd", "dm")
    xparts = ("8", "16") if hyb else ("",)
    for q in range(NQ):
        if q >= 2:
            sp.append(("wait", f"mask:{q - 2}:3"))
        for w in whiches:
            sp.append(("dmaw", w, q, f"ld:w:{q}:{w}"))
        bb = q
        if bb < nbt:
            if bb >= 2:
                sp.append(("wait", f"cast:{bb - 2}"))
            for xp in xparts:
                sp.append(("dmax", bb, xp, f"ld:x:{bb}:{xp}"))
        if q == 0:
            # consts are off the critical path (first consumer: the ACT
            # eviction of chunk 0, and DVE wsm/wblk before s2(0)); issuing
            # them after the q0 weights + x0 starts the masked-weight /
            # cast chain ~3us earlier.
            sp.append(("dmac", "cst", "ld:c:cst"))
    for bb in range(NQ, nbt):
        sp.append(("wait", f"cast:{bb - 2}"))
        for xp in xparts:
            sp.append(("dmax", bb, xp, f"ld:x:{bb}:{xp}"))
        st = bb - NQ
        sp.append(("wait", f"final:{st}:1"))
        sp.append(("dmao", st, f"st:{st}"))
    for st in range(max(0, nbt - NQ), nbt):
        sp.append(("wait", f"final:{st}:1"))
        sp.append(("dmao", st, f"st:{st}"))
    sp.append(("waitalldout",))

    dve = []
    dve.append(("wait", "wqdone:0"))
    for g in range(4):
        dve.append(("mask", 0, g, f"mask:0:{g}"))
    dve.append(("wait", "cdone"))
    dve.append(("wsm", "wsm"))
    dve.append(("wblkms", "wblkms"))
    # DVE is deeply pipelined: reading wsm_t/wblk back-to-back on the same
    # engine needs an explicit drain via a self-semaphore wait.
    dve.append(("wait", "wblkms"))
    for cc in range(NCH):
        dve.append(("wblk", cc, f"wblk:{cc}"))
    dve.append(("wait", "xgdone:0"))
    if hyb:
        dve.append(("cast8", 0, "cast8:0"))
    dve.append(("cast", 0, "cast:0"))
    for q in range(1, NQ):
        dve.append(("wait", f"wqdone:{q}"))
        for g in range(4):
            dve.append(("mask", q, g, f"mask:{q}:{g}"))
    for bb in range(1, nbt):
        if bb >= 2:
            dve.append(("wait", f"mm8:{NCH * (bb - 2) + NCH - 1}:1"))
        dve.append(("wait", f"xgdone:{bb}"))
        if hyb:
            dve.append(("cast8", bb, f"cast8:{bb}"))
        dve.append(("cast", bb, f"cast:{bb}"))

    act = [("wait", "cdone")]
    for k in range(k_tot):
        bb, cc = divmod(k, NCH)
        # PSUM reads must not cross bank boundaries: evict each 512-wide
        # half separately (same soma chunk => same bias for both halves).
        for half in range(2):
            act.append(("wait", f"mm8:{k}:{half}"))
            act.append(("evict", k, half, f"evict:{k}:{half}"))
        if cc == NCH - 1:
            for half in range(2):
                act.append(("wait", f"s2:{k}:{half}"))
                if half == 0 and bb >= 2:
                    # osb[bb%2] must be fully drained by the bb-2 store
                    # before either half is overwritten.
                    act.append(("wait", f"st:{bb - 2}"))
                act.append(("final", bb, half, f"final:{bb}:{half}"))

    pe = []

    def _emit_s2(j, tail=False):
        jb = j // NCH
        if j % NCH == 0 and jb >= 2:
            pe.append(("wait", f"final:{jb - 2}:1"))
        if j == 0:
            pe.append(("wait", f"wblk:{NCH - 1}"))
        if tail:
            pe.append(("wait", f"evict:{j}:1"))
        pe.append(("s2", j, 0, f"s2:{j}:0"))
        pe.append(("s2", j, 1, f"s2:{j}:1"))

    for k in range(k_tot):
        bb, cc = divmod(k, NCH)
        if bb == 0 and cc % 4 == 0:
            pe.append(("wait", f"mask:{cc // 4}:3"))
        if cc == 0:
            pe.append(("wait", f"cast:{bb}"))
        if k >= 2:
            pe.append(("wait", f"evict:{k - 2}:1"))
        if hyb:
            # all DoubleRow matmuls first (both subs), then all bf16: the
            # PE pays a pipeline break per weight-mode switch, so batch
            # same-mode matmuls.  mm8 events fire on the bf16 tails.
            pe.append(("mmdr", k, 0))
            pe.append(("mmdr", k, 1))
            pe.append(("mmbf", k, 0, f"mm8:{k}:0"))
            pe.append(("mmbf", k, 1, f"mm8:{k}:1"))
        else:
            pe.append(("mm4", k, 0, f"mm8:{k}:0"))
            pe.append(("mm4", k, 1, f"mm8:{k}:1"))
        if k >= 2:
            _emit_s2(k - 2)
    for j in (k_tot - 2, k_tot - 1):
        _emit_s2(j, tail=True)

    return {"sp": sp, "dve": dve, "act": act, "pe": pe}


def _plan_events(streams, repeat: int = 1):
    """Assign each event its (sem_key, value-after-inc)."""
    events = {}
    counts: dict = {}

    def bump(sem, inc):
        counts[sem] = counts.get(sem, 0) + inc
        return counts[sem]

    for eng, ops in streams.items():
        for op in ops:
            kind = op[0]
            if kind in ("wait", "waitalldout", "mmdr"):
                continue
            ev = op[-1]
            if kind == "dmac":
                events[ev] = ("c", bump("c", 16))
            elif kind == "dmaw":
                q = op[2]
                events[ev] = (f"w{q % 2}", bump(f"w{q % 2}", 16))
            elif kind == "dmax":
                bb = op[1]
                events[ev] = (f"x{bb % 2}", bump(f"x{bb % 2}", 16))
            elif kind == "cast8":
                events[ev] = ("dve", bump("dve", 1))
            elif kind == "dmao":
                st = op[1]
                events[ev] = (f"do{st % 2}", bump(f"do{st % 2}", 16))
            elif eng == "dve":
                events[ev] = ("dve", bump("dve", 1))
            elif eng == "pe":
                events[ev] = ("pe", bump("pe", 1))
            elif eng == "act":
                events[ev] = ("act", bump("act", 1))
            else:
                raise ValueError((eng, kind))
    events["cdone"] = ("c", counts["c"])
    hyb = any(op[0] == "cast8" for op in streams["dve"])
    for q in range(NQ):
        events[f"wqdone:{q}"] = events[
            f"ld:w:{q}:dm16" if hyb else f"ld:w:{q}:dm"]
    for bb in range(repeat * NB):
        events[f"xgdone:{bb}"] = events[
            f"ld:x:{bb}:16" if hyb else f"ld:x:{bb}:"]
    events["_dout_totals"] = (counts.get("do0", 0), counts.get("do1", 0))
    return events


def build_program(mm_mode: str = "bf16", leaky_mode: str = "act",
                  repeat: int = 1, variant: str = "full"):
    import concourse.bass as bass
    import concourse.mybir as mybir

    key = (mm_mode, repeat)
    if key in _PROGRAM_CACHE:
        return _PROGRAM_CACHE[key]

    f32 = mybir.dt.float32
    u32 = mybir.dt.uint32
    bf16 = mybir.dt.bfloat16
    fp8 = mybir.dt.float8e4
    hyb = mm_mode == "hyb"
    mm_dt = fp8 if mm_mode == "fp8" else bf16
    evict_scale = (1.0 / 32.0) if mm_mode in ("fp8", "hyb") else 1.0
    KT8 = KT // 2  # hyb: k-chunks 0..3 fp8 (DoubleRow), 4..7 bf16
    mult = mybir.AluOpType.mult
    prelu = mybir.ActivationFunctionType.Prelu
    DR = mybir.MatmulPerfMode.DoubleRow

    nc = bass.Bass("TRN2")

    # DRAM.  All layouts are host-prearranged so every DMA is a large
    # per-partition-contiguous transfer.
    if hyb:
        xR8 = nc.dram_tensor("xR8", [NB, P, KT8, BBLK], fp8,
                             kind="ExternalInput")
        xR16 = nc.dram_tensor("xR16", [NB, P, KT8, BBLK], bf16,
                              kind="ExternalInput")
        wdR8 = nc.dram_tensor("wdR8", [NQ, P, KT8, WPC], fp8,
                              kind="ExternalInput")
        dmR8 = nc.dram_tensor("dmR8", [NQ, P, KT8, WPC], fp8,
                              kind="ExternalInput")
        wdR16 = nc.dram_tensor("wdR16", [NQ, P, KT8, WPC], bf16,
                               kind="ExternalInput")
        dmR16 = nc.dram_tensor("dmR16", [NQ, P, KT8, WPC], bf16,
                               kind="ExternalInput")
    else:
        xR = nc.dram_tensor("xR", [NB, P, KT, BBLK], mm_dt,
                            kind="ExternalInput")
        wdR = nc.dram_tensor("wdR", [NQ, P, KT, WPC], mm_dt,
                             kind="ExternalInput")
        dmR = nc.dram_tensor("dmR", [NQ, P, KT, WPC], mm_dt,
                             kind="ExternalInput")
    # merged consts: [bd | wsd | smd | bs | gmk] = 16+16+16+1+8 = 57 cols
    cst = nc.dram_tensor("cst", [P, 57], f32, kind="ExternalInput")
    outR = nc.dram_tensor("outR", [NB, P, BBLK], f32, kind="ExternalOutput")
    dram_in = {"cst": cst}

    # SBUF
    if hyb:
        wm8 = nc.alloc_sbuf_tensor("wm8", [P, NQ, KT8, WPC], fp8)
        wm16 = nc.alloc_sbuf_tensor("wm16", [P, NQ, KT8, WPC], bf16)
        w8_st = {w: [nc.alloc_sbuf_tensor(f"{w}st{i}", [P, KT8, WPC], fp8)
                     for i in range(2)] for w in ("wd8", "dm8")}
        w16_st = {w: [nc.alloc_sbuf_tensor(f"{w}st{i}", [P, KT8, WPC], bf16)
                      for i in range(2)] for w in ("wd16", "dm16")}
        xall8 = [nc.alloc_sbuf_tensor(f"xall8_{i}", [P, KT8, BBLK], fp8)
                 for i in range(2)]
        xc8 = [nc.alloc_sbuf_tensor(f"xc8_{i}", [P, KT8, BBLK], fp8)
               for i in range(2)]
        xall = [nc.alloc_sbuf_tensor(f"xall{i}", [P, KT8, BBLK], bf16)
                for i in range(2)]
        xc = [nc.alloc_sbuf_tensor(f"xc{i}", [P, KT8, BBLK], bf16)
              for i in range(2)]
    else:
        wm2 = nc.alloc_sbuf_tensor("wm2", [P, NQ, KT, WPC], mm_dt)
        wd_st = [nc.alloc_sbuf_tensor(f"wdst{i}", [P, KT, WPC], mm_dt)
                 for i in range(2)]
        dm_st = [nc.alloc_sbuf_tensor(f"dmst{i}", [P, KT, WPC], mm_dt)
                 for i in range(2)]
        xall = [nc.alloc_sbuf_tensor(f"xall{i}", [P, KT, BBLK], mm_dt)
                for i in range(2)]
        xc = [nc.alloc_sbuf_tensor(f"xc{i}", [P, KT, BBLK], mm_dt)
              for i in range(2)]
    hT = [nc.alloc_sbuf_tensor(f"hT{i}", [P, BBLK], bf16) for i in range(NH)]
    wblk = nc.alloc_sbuf_tensor("wblk", [P, NCH, P], bf16)
    osb = [nc.alloc_sbuf_tensor(f"osb{i}", [P, BBLK], f32) for i in range(2)]
    cst_t = nc.alloc_sbuf_tensor("cst_t", [P, 57], f32)
    bd_t = cst_t[:, 0:NCH]
    wsd_t = cst_t[:, NCH: 2 * NCH]
    smd_t = cst_t[:, 2 * NCH: 3 * NCH]
    bs_t = cst_t[:, 3 * NCH: 3 * NCH + 1]
    g_t = cst_t[:, 3 * NCH + 1: 3 * NCH + 9]
    wsm_t = nc.alloc_sbuf_tensor("wsm_t", [P, NCH], f32)
    sb_in = {"cst": cst_t}

    # PSUM: all single-bank tensors (proven-stable pattern): 4 rotating
    # stage-1 banks + 2 subs x 2 parities of stage-2 banks = all 8 banks
    ph = [nc.alloc_psum_tensor(f"ph{i}", [P, 512], f32) for i in range(4)]
    pout = [[nc.alloc_psum_tensor(f"pout{s}_{i}", [P, 512], f32)
             for i in range(2)] for s in range(2)]

    streams = _streams(repeat, hyb)
    events = _plan_events(streams, repeat)
    dout_totals = events["_dout_totals"]

    def run_stream(eng_api, ops, sems, waited):
        def wait(ev):
            sem_key, val = events[ev]
            if waited.get(sem_key, -1) >= val:
                return
            waited[sem_key] = val
            eng_api.wait_ge(sems[sem_key], val)

        def inc_of(ev):
            return sems[events[ev][0]]

        for op in ops:
            kind = op[0]
            if kind == "wait":
                wait(op[1])
            elif kind == "waitalldout":
                eng_api.wait_ge(sems["do0"], dout_totals[0])
                eng_api.wait_ge(sems["do1"], dout_totals[1])
            elif kind == "dmac":
                name, ev = op[1], op[2]
                eng_api.dma_start(sb_in[name][:], dram_in[name][:]).then_inc(
                    inc_of(ev), 16)
            elif kind == "dmaw":
                which, q, ev = op[1], op[2], op[3]
                if hyb:
                    dmap = {"wd8": (w8_st["wd8"], wdR8),
                            "dm8": (w8_st["dm8"], dmR8),
                            "wd16": (w16_st["wd16"], wdR16),
                            "dm16": (w16_st["dm16"], dmR16)}
                    dsts, srct = dmap[which]
                    eng_api.dma_start(
                        dsts[q % 2][:], srct[q]).then_inc(inc_of(ev), 16)
                else:
                    dst = (wd_st if which == "wd" else dm_st)[q % 2]
                    src = (wdR if which == "wd" else dmR)
                    eng_api.dma_start(dst[:], src[q]).then_inc(inc_of(ev), 16)
            elif kind == "dmax":
                bb, xp, ev = op[1], op[2], op[3]
                if hyb and xp == "8":
                    eng_api.dma_start(
                        xall8[bb % 2][:], xR8[bb % NB]).then_inc(
                        inc_of(ev), 16)
                elif hyb:
                    eng_api.dma_start(
                        xall[bb % 2][:], xR16[bb % NB]).then_inc(
                        inc_of(ev), 16)
                else:
                    eng_api.dma_start(
                        xall[bb % 2][:], xR[bb % NB]).then_inc(inc_of(ev), 16)
            elif kind == "dmao":
                st, ev = op[1], op[2]
                eng_api.dma_start(
                    outR[st % NB], osb[st % 2][:]).then_inc(inc_of(ev), 16)
            elif kind == "mask":
                q, g, ev = op[1], op[2], op[3]
                if hyb:
                    # g 0,1 -> fp8 pair-planes; g 2,3 -> bf16 pair-planes
                    if g < 2:
                        dst, a, b = (wm8, w8_st["wd8"], w8_st["dm8"])
                        gg = g
                    else:
                        dst, a, b = (wm16, w16_st["wd16"], w16_st["dm16"])
                        gg = g - 2
                    nc.vector.tensor_tensor(
                        dst[:, q, 2 * gg: 2 * gg + 2, :],
                        a[q % 2][:, 2 * gg: 2 * gg + 2, :],
                        b[q % 2][:, 2 * gg: 2 * gg + 2, :],
                        mult,
                    ).then_inc(inc_of(ev), 1)
                else:
                    nc.vector.tensor_tensor(
                        wm2[:, q, 2 * g: 2 * g + 2, :],
                        wd_st[q % 2][:, 2 * g: 2 * g + 2, :],
                        dm_st[q % 2][:, 2 * g: 2 * g + 2, :],
                        mult,
                    ).then_inc(inc_of(ev), 1)
            elif kind == "cast8":
                bb, ev = op[1], op[2]
                nc.vector.tensor_copy(
                    xc8[bb % 2][:].bitcast(u32), xall8[bb % 2][:].bitcast(u32)
                ).then_inc(inc_of(ev), 1)
            elif kind == "cast":
                bb, ev = op[1], op[2]
                if mm_mode == "fp8":
                    # fp8 has no DVE packing; a u32 bitcast copy moves 4
                    # elements per lane-op instead of 1.
                    nc.vector.tensor_copy(
                        xc[bb % 2][:].bitcast(u32), xall[bb % 2][:].bitcast(u32)
                    ).then_inc(inc_of(ev), 1)
                else:
                    # bf16 contiguous copy runs in DVE 4x mode.
                    nc.vector.tensor_copy(
                        xc[bb % 2][:], xall[bb % 2][:]
                    ).then_inc(inc_of(ev), 1)
            elif kind == "wsm":
                nc.vector.tensor_tensor(
                    wsm_t[:], wsd_t, smd_t, mult
                ).then_inc(inc_of(op[1]), 1)
            elif kind == "wblkms":
                nc.vector.memset(wblk[:], 0.0).then_inc(inc_of(op[1]), 1)
            elif kind == "wblk":
                cc, ev = op[1], op[2]
                nc.vector.tensor_scalar_mul(
                    wblk[:, cc, 8 * cc: 8 * cc + 8], g_t,
                    wsm_t[:, cc: cc + 1],
                ).then_inc(inc_of(ev), 1)
            elif kind == "mm4":
                k, sub, ev = op[1], op[2], op[3]
                bb, cc = divmod(k, NCH)
                q, c4 = divmod(cc, 4)
                dst = ph[(2 * k + sub) % 4][:]
                if mm_mode == "fp8":
                    for jj in range(KT2):
                        ins = nc.tensor.matmul(
                            dst,
                            wm2[:, q, 2 * jj: 2 * jj + 2, bass.ts(c4, P)],
                            xc[bb % 2][:, 2 * jj: 2 * jj + 2,
                                       bass.ts(sub, 512)],
                            start=(jj == 0),
                            stop=(jj == KT2 - 1),
                            perf_mode=DR,
                        )
                else:
                    for j in range(KT):
                        ins = nc.tensor.matmul(
                            dst,
                            wm2[:, q, j, bass.ts(c4, P)],
                            xc[bb % 2][:, j, bass.ts(sub, 512)],
                            start=(j == 0),
                            stop=(j == KT - 1),
                        )
                ins.then_inc(inc_of(ev), 1)
            elif kind == "mmdr":
                k, sub = op[1], op[2]
                bb, cc = divmod(k, NCH)
                q, c4 = divmod(cc, 4)
                dst = ph[(2 * k + sub) % 4][:]
                for jj in range(KT8 // 2):
                    nc.tensor.matmul(
                        dst,
                        wm8[:, q, 2 * jj: 2 * jj + 2, bass.ts(c4, P)],
                        xc8[bb % 2][:, 2 * jj: 2 * jj + 2,
                                    bass.ts(sub, 512)],
                        start=(jj == 0),
                        stop=False,
                        perf_mode=DR,
                    )
            elif kind == "mmbf":
                k, sub, ev = op[1], op[2], op[3]
                bb, cc = divmod(k, NCH)
                q, c4 = divmod(cc, 4)
                dst = ph[(2 * k + sub) % 4][:]
                for j in range(KT8):
                    ins = nc.tensor.matmul(
                        dst,
                        wm16[:, q, j, bass.ts(c4, P)],
                        xc[bb % 2][:, j, bass.ts(sub, 512)],
                        start=False,
                        stop=(j == KT8 - 1),
                    )
                ins.then_inc(inc_of(ev), 1)
            elif kind == "s2":
                j, sub, ev = op[1], op[2], op[3]
                jb, cc = divmod(j, NCH)
                nc.tensor.matmul(
                    pout[sub][jb % 2][:],
                    wblk[:, cc, :],
                    hT[j % NH][:, bass.ts(sub, 512)],
                    start=(cc == 0),
                    stop=(cc == NCH - 1),
                ).then_inc(inc_of(ev), 1)
            elif kind == "evict":
                k, half, ev = op[1], op[2], op[3]
                bb, cc = divmod(k, NCH)
                nc.scalar.activation(
                    hT[k % NH][:, bass.ts(half, 512)],
                    ph[(2 * k + half) % 4][:], prelu,
                    bias=bd_t[:, cc: cc + 1], scale=evict_scale, alpha=SLOPE,
                ).then_inc(inc_of(ev), 1)
            elif kind == "final":
                bb, half, ev = op[1], op[2], op[3]
                nc.scalar.activation(
                    osb[bb % 2][:, bass.ts(half, 512)],
                    pout[half][bb % 2][:], prelu,
                    bias=bs_t, scale=1.0, alpha=SLOPE,
                ).then_inc(inc_of(ev), 1)
            else:
                raise ValueError(kind)

    from contextlib import ExitStack

    with ExitStack() as es:
        sems = {
            key: es.enter_context(nc.semaphore(f"sem_{key}"))
            for key in ("c", "w0", "w1", "x0", "x1", "do0", "do1",
                        "dve", "pe", "act")
        }
        block = es.enter_context(nc.Block())

        @block.sync
        def _(sync):
            run_stream(sync, streams["sp"], sems, {})

        @block.vector
        def _(vector):
            run_stream(vector, streams["dve"], sems, {})

        @block.scalar
        def _(scalar):
            run_stream(scalar, streams["act"], sems, {})

        @block.tensor
        def _(tensor):
            run_stream(tensor, streams["pe"], sems, {})

    _PROGRAM_CACHE[key] = nc
    return nc


def _neuron_perm():
    """Stage-2 output partition for neuron n (identity for full-width s2)."""
    return np.arange(N_SH)


def make_in_maps(x, Wd, bd, Ws, bs, dendrite_mask, soma_mask, mm_mode="bf16"):
    """Host-side sharding.  Layout + dtype-quantization transforms only:
    all reference arithmetic (masking, matmuls, bias, activations) runs on
    device.  Wd ships pre-scaled by 32 in fp8 mode (exact power of two,
    compensated by the on-device eviction scale=1/32)."""
    f32 = np.float32
    x = np.asarray(x, f32)
    Wd = np.asarray(Wd, f32)
    bd = np.asarray(bd, f32)
    Ws = np.asarray(Ws, f32)
    bs = np.asarray(bs, f32)
    dendrite_mask = np.asarray(dendrite_mask, f32)
    soma_mask = np.asarray(soma_mask, f32)

    import ml_dtypes

    f8 = ml_dtypes.float8_e4m3
    b16 = ml_dtypes.bfloat16
    hyb = mm_mode == "hyb"
    if mm_mode == "fp8":
        dt = f8
        wscale = 32.0
    elif hyb:
        dt = None  # split below: k-chunks 0..3 fp8, 4..7 bf16, both x32
        wscale = 32.0
    else:
        dt = b16
        wscale = 1.0

    # xR[bbp, p, j, n] = x[bb*BBLK + n, 128j + p]
    xRf = x.reshape(NB, BBLK, KT, P).transpose(0, 3, 2, 1)
    if hyb:
        xR8v = np.ascontiguousarray(xRf[:, :, : KT // 2].astype(f8))
        xR16v = np.ascontiguousarray(xRf[:, :, KT // 2:].astype(b16))
    else:
        xRv = np.ascontiguousarray(xRf.astype(dt))

    # diagonal (per-neuron) slices of the soma weights / mask
    nn_i = np.arange(N_NEURONS)[:, None]
    dd_i = ND * np.arange(N_NEURONS)[:, None] + np.arange(ND)[None, :]
    ws_diag = Ws[nn_i, dd_i]                            # [N_NEURONS, 16]
    sm_diag = soma_mask[nn_i, dd_i]                     # [N_NEURONS, 16]
    # soma_mask must be supported only on the block diagonal (it is, by
    # construction); verify cheaply so we never silently drop weight.
    assert np.count_nonzero(soma_mask) == np.count_nonzero(sm_diag), (
        "soma_mask has off-block-diagonal support; kernel sharding invalid"
    )
    wflat = ws_diag.reshape(-1)                         # [N_SOMA], soma order
    sflat = sm_diag.reshape(-1)

    gmkv = (np.arange(P)[:, None] // ND == np.arange(8)[None, :]).astype(f32)

    in_maps = []
    PI = _neuron_perm()
    for c in range(N_CORES):
        sl = slice(c * S_SH, (c + 1) * S_SH)
        nl = slice(c * N_SH, (c + 1) * N_SH)
        bs_perm = np.empty(N_SH, f32)
        bs_perm[PI] = bs[nl]
        # wdR[q, p, j, sw] = wscale * Wd[c*2048 + 512q + sw, 128j + p]
        wdc = (Wd[sl] * wscale).reshape(NQ, WPC, KT, P).transpose(0, 3, 2, 1)
        dmc = dendrite_mask[sl].reshape(NQ, WPC, KT, P).transpose(0, 3, 2, 1)
        cstv = np.ascontiguousarray(np.concatenate([
            bd[sl].reshape(NCH, P).T,
            wflat[sl].reshape(NCH, P).T,
            sflat[sl].reshape(NCH, P).T,
            bs_perm.reshape(N_SH, 1),
            gmkv,
        ], axis=1).astype(f32))
        if hyb:
            tensors = {
                "xR8": xR8v,
                "xR16": xR16v,
                "wdR8": np.ascontiguousarray(wdc[:, :, : KT // 2].astype(f8)),
                "dmR8": np.ascontiguousarray(dmc[:, :, : KT // 2].astype(f8)),
                "wdR16": np.ascontiguousarray(
                    wdc[:, :, KT // 2:].astype(b16)),
                "dmR16": np.ascontiguousarray(
                    dmc[:, :, KT // 2:].astype(b16)),
            }
        else:
            tensors = {
                "xR": xRv,
                "wdR": np.ascontiguousarray(wdc.astype(dt)),
                "dmR": np.ascontiguousarray(dmc.astype(dt)),
            }
        in_maps.append({**tensors, "cst": cstv})
    return in_maps


def _assemble_out(outR_cores):
    """outR_cores: list of [NB, P, BBLK] f32 per core -> [B, N_NEURONS]."""
    PI = _neuron_perm()
    cols = [
        np.asarray(o, np.float32)[:, PI, :].transpose(0, 2, 1).reshape(B, P)
        for o in outR_cores
    ]
    return np.ascontiguousarray(np.concatenate(cols, axis=1))


def run(inputs, trace=False, mm_mode="bf16", leaky_mode="act"):
    """Build, compile and execute on 8 NeuronCores; returns (out, results)."""
    from concourse.bass_utils import run_bass_kernel_spmd

    nc = build_program(mm_mode)
    in_maps = make_in_maps(**inputs, mm_mode=mm_mode)
    res = run_bass_kernel_spmd(nc, in_maps, list(range(N_CORES)), trace=trace)
    out = _assemble_out([res.results[c]["outR"] for c in range(N_CORES)])
    return out, res


def kernel(**inputs) -> np.ndarray:
    return run(inputs)[0]


def bench(inputs, iters=20, warmup=3, mm_mode="bf16", leaky_mode="act",
          repeat=1, variant="full"):
    """Time repeated on-device executions of the compiled program.

    Mirrors bass2jax.run_bass_via_pjrt's multi-core path, but keeps the
    jitted executable and device-resident inputs so per-iteration wall time
    = dispatch overhead + NEFF execution.  Returns (times_s, out).
    """
    import time

    import jax
    import numpy as np
    from jax.sharding import Mesh, PartitionSpec
    from jax.experimental.shard_map import shard_map

    from concourse import bass2jax
    from concourse import mybir

    bass2jax.install_neuronx_cc_hook()
    nc = build_program(mm_mode, repeat=repeat)
    if not nc.is_finalized():
        nc.finalize()
    in_maps = make_in_maps(**inputs, mm_mode=mm_mode)

    partition_name = (
        nc.partition_id_tensor.name if nc.partition_id_tensor else None
    )
    in_names: list[str] = []
    out_names: list[str] = []
    out_avals = []
    zero_outs = []
    for alloc in nc.m.functions[0].allocations:
        if not isinstance(alloc, mybir.MemoryLocationSet):
            continue
        name = alloc.memorylocations[0].name
        if alloc.kind == "ExternalInput":
            if name != partition_name:
                in_names.append(name)
        elif alloc.kind == "ExternalOutput":
            out_names.append(name)
            shape = tuple(alloc.tensor_shape)
            dtype = mybir.dt.np(alloc.dtype)
            out_avals.append(jax.core.ShapedArray(shape, dtype))
            zero_outs.append(np.zeros(shape, dtype))
    n_params = len(in_names)
    all_in_names = list(in_names) + list(out_names)
    if partition_name is not None:
        all_in_names.append(partition_name)

    def _body(*args):
        operands = list(args)
        if partition_name is not None:
            operands.append(bass2jax.partition_id_tensor())
        outs = bass2jax._bass_exec_p.bind(
            *operands,
            out_avals=tuple(out_avals),
            in_names=tuple(all_in_names),
            out_names=tuple(out_names),
            lowering_input_output_aliases=(),
            sim_require_finite=True,
            sim_require_nnan=True,
            nc=nc,
        )
        return tuple(outs)

    devices = jax.devices()[:N_CORES]
    mesh = Mesh(np.asarray(devices), ("core",))
    nin = n_params + len(out_names)
    fn = jax.jit(
        shard_map(
            _body,
            mesh=mesh,
            in_specs=(PartitionSpec("core"),) * nin,
            out_specs=(PartitionSpec("core"),) * len(out_names),
            check_rep=False,
        ),
        keep_unused=True,
    )
    concat_in = [
        np.concatenate([np.asarray(in_maps[c][n]) for c in range(N_CORES)], 0)
        for n in in_names
    ]
    concat_zero = [
        np.zeros((N_CORES * z.shape[0], *z.shape[1:]), z.dtype)
        for z in zero_outs
    ]
    dev_args = [jax.device_put(a) for a in (*concat_in, *concat_zero)]
    for _ in range(warmup):
        r = fn(*dev_args)
        jax.block_until_ready(r)
    times = []
    for _ in range(iters):
        t0 = time.perf_counter()
        r = fn(*dev_args)
        jax.block_until_ready(r)
        times.append(time.perf_counter() - t0)
    outR_all = np.asarray(r[0]).reshape(N_CORES, NB, P, BBLK)
    out = _assemble_out([outR_all[c] for c in range(N_CORES)])
    return times, out
